# revision 18
# baseline (speedup 1.0000x reference)
"""Causal self-attention (QK-RMSNorm + RoPE) Trainium2 kernel.

Sharding (Megatron-style, per the TP-over-heads hint):
  8 cores = 2 (batch) x 4 (head groups of 4 heads).
  Each core computes qkv/attention for its 4 heads on its batch and a partial
  projection output; the host sums the 4 partials per batch (the "all-reduce")
  and transposes (the device emits the output feature-major).

Per-core pipeline (all matmuls bf16/fp16 with fp32 PSUM accumulation):
  x^T arrives pre-transposed + bf16 from the host, streamed in t-quarters.
  qkv = W_shard @ x^T (token-major PSUM) in pass order v, q, k; for q/k:
  fused RMSNorm (one ACT square + DVE segmented reduce + sqrt/recip + ACT
  per-head scale) + RoPE (norm weights pre-folded into the rope tables),
  PE-transpose to [d, t] layout.
  The k-pass is emitted in four t-blocks interleaved with the attention
  q-blocks (block j only needs k tiles 0..4j+3), so the exp-bound attention
  sections overlap the matmul-bound k-pass.
  Attention per (q-block j, head): scores^T = k^T.T @ q^T, exp on ACT with
  tiles paired into 2-bank PSUM (no max subtraction needed: |scores| <=
  sqrt(hd); exp carries a 1/16 bias for fp16 range), causal mask by tile
  skipping + 4 diagonal masks (fp16, DVE 2x), y^T = v.T @ p^T in fp16,
  denominator via fp16 DVE accumulate + fp16 ones-matmul + DVE reciprocal +
  K=1 matmul broadcast; the projection for each block runs interleaved to
  overlap later attention/k-pass work.
"""

import math
from contextlib import ExitStack

import numpy as np
import ml_dtypes

import concourse.bass as bass
import concourse.mybir as mybir
import concourse.tile as tile
from concourse import bacc

F32 = mybir.dt.float32
BF16 = mybir.dt.bfloat16
F16 = mybir.dt.float16
AF = mybir.ActivationFunctionType

# Problem constants (hardcoded; kernel.py must be self-contained)
B, T, C, H, HD = 2, 2048, 2048, 16, 128
N_CORES = 8
DP = 2                 # data-parallel ways (batch)
TPW = N_CORES // DP    # tensor-parallel ways (head groups)
HG = H // TPW          # heads per core
EPS = 1e-6
EXP_BIAS = -math.log(16.0)  # scale exp by 1/16: cancels in softmax, keeps
                            # the fp16 p/denominator chain far from overflow


def build_nc(T_=T, C_=C, HG_=HG, hd=HD, TQ=512):
    NT = T_ // 128       # token tiles
    NCt = C_ // 128      # contraction tiles for qkv
    NJ = T_ // TQ        # query-block tiles
    NO = C_ // 128       # output feature tiles
    R = TQ // 128        # diagonal mask patterns per query block
    F1 = HG_ * hd        # width of one of q/k/v chunks on this core
    HB = hd // 2
    TL = T_ // NJ        # tokens per x^T quarter (= TQ)
    sm_scale = 1.0 / math.sqrt(hd)

    nc = bacc.Bacc(None, target_bir_lowering=False)
    xT = nc.dram_tensor("xT", [C_, T_], BF16, kind="ExternalInput")
    wqkvT = nc.dram_tensor("wqkvT", [C_, 3 * F1], BF16, kind="ExternalInput")
    wprojT = nc.dram_tensor("wprojT", [F1, C_], BF16, kind="ExternalInput")
    rope_q = nc.dram_tensor("rope_q", [T_, 4 * HB], BF16, kind="ExternalInput")
    rope_k = nc.dram_tensor("rope_k", [T_, 4 * HB], BF16, kind="ExternalInput")
    masks_d = nc.dram_tensor("masks", [R * 128, TQ], F16, kind="ExternalInput")
    ident_d = nc.dram_tensor("ident", [128, 128], BF16, kind="ExternalInput")
    outT = nc.dram_tensor("outT", [C_, T_], BF16, kind="ExternalOutput")

    with tile.TileContext(nc) as tc, ExitStack() as big:
        persist = big.enter_context(tc.tile_pool(name="persist", bufs=1))
        v_all = persist.tile([128, NT, F1], F16, tag="v_all")
        qkT = persist.tile([128, 2, HG_, T_], BF16, tag="qkT")
        ident = persist.tile([128, 128], BF16, tag="ident")
        nc.sync.dma_start(ident, ident_d[:])
        eps_t = persist.tile([128, 1], F32, tag="eps")
        nc.vector.memset(eps_t, EPS)
        ebias_t = persist.tile([128, 1], F32, tag="ebias")
        nc.vector.memset(ebias_t, EXP_BIAS)
        ones_h = persist.tile([128, 1], F16, tag="ones_h")
        nc.vector.memset(ones_h, 1.0)
        ones_r = persist.tile([1, 128], BF16, tag="ones_r")
        nc.vector.memset(ones_r, 1.0)
        masks_t = persist.tile([128, R * TQ], F16, tag="masks")
        wp = persist.tile([128, HG_, C_], BF16, tag="wp")

        xh_pool = big.enter_context(tc.tile_pool(name="xq", bufs=2))
        wt_pool = big.enter_context(tc.tile_pool(name="wt", bufs=2))
        rope_pool = big.enter_context(tc.tile_pool(name="rope", bufs=1))
        scr = big.enter_context(tc.tile_pool(name="qkscr", bufs=3))
        pP = big.enter_context(tc.tile_pool(name="pP", bufs=8))
        dP = big.enter_context(tc.tile_pool(name="dP", bufs=2))
        rbP = big.enter_context(tc.tile_pool(name="rbP", bufs=2))
        yP = big.enter_context(tc.tile_pool(name="yP", bufs=2))
        oP = big.enter_context(tc.tile_pool(name="oP", bufs=3))
        # PSUM: 2 + 1 + 2 + 1 + 2 = 8 banks
        ps_qkv = big.enter_context(tc.tile_pool(name="ps_qkv", bufs=2, space="PSUM"))
        ps_tr = big.enter_context(tc.tile_pool(name="ps_tr", bufs=1, space="PSUM"))
        ps_s = big.enter_context(tc.tile_pool(name="ps_s", bufs=1, space="PSUM"))
        ps_yp = big.enter_context(tc.tile_pool(name="ps_y", bufs=1, space="PSUM"))
        ps_m = big.enter_context(tc.tile_pool(name="ps_m", bufs=2, space="PSUM"))

        rope_sb = {}

        def load_rope(nm, dr):
            t_ = rope_pool.tile([128, NT, 4 * HB], BF16, tag=f"rope{nm}")
            nc.gpsimd.dma_start(t_, dr[:].rearrange("(n p) f -> p n f", p=128))
            rope_sb[nm] = t_

        # W chunks on the sync HWDGE ring; the first (v) pass is gated by W
        # arrival, so split its chunks across the sync + gpsimd rings
        def load_wt(fc, split=False):
            wt = []
            for ci in range(NCt):
                t_ = wt_pool.tile([128, F1], BF16, tag=f"wt{ci}", bufs=2)
                eng = nc.gpsimd if (split and ci % 2 == 1) else nc.sync
                eng.dma_start(
                    t_,
                    wqkvT[ci * 128:(ci + 1) * 128, fc * F1:(fc + 1) * F1],
                )
                wt.append(t_)
            return wt

        # x^T [c, t] staged by the host, streamed as t-quarters (two
        # half-loads each on separate rings to halve the arrival latency)
        xT_r = xT[:].rearrange("(ci p) t -> p ci t", p=128)
        xq_rings = [nc.scalar, nc.sync]

        def load_xq(qt):
            t_ = xh_pool.tile([128, NCt, TL], BF16, tag="xq")
            for hf in range(2):
                sl = slice(qt * TL + hf * (TL // 2), qt * TL + (hf + 1) * (TL // 2))
                dl = slice(hf * (TL // 2), (hf + 1) * (TL // 2))
                xq_rings[hf].dma_start(t_[:, :, dl], xT_r[:, :, sl])
            return t_

        # ---------------- token-major QKV tile (shared by all passes) ------
        def qkv_tile(fc, i, wt, xq):
            ts = slice(i * 128, (i + 1) * 128)
            lt = slice((i % NJ) * 128, (i % NJ + 1) * 128)
            ps = ps_qkv.tile([128, F1], F32, tag="psqkv")
            for ci in range(NCt):
                nc.tensor.matmul(
                    ps,
                    xq[:, ci, lt],
                    wt[ci],
                    start=(ci == 0),
                    stop=(ci == NCt - 1),
                )
            if fc == 2:
                nc.vector.tensor_copy(v_all[:, i, :], ps)
                return
            rp = rope_sb["q" if fc == 0 else "k"]
            # RMS norm stats: one ACT square over the whole tile, per-head
            # sums on DVE, sqrt on ACT + reciprocal on DVE
            sq = scr.tile([128, F1], BF16, tag="sq")
            nc.scalar.activation(sq, ps, AF.Square)
            sq3 = sq.rearrange("p (h d) -> p h d", h=HG_)
            ssq = scr.tile([128, HG_], F32, tag="ssq")
            nc.vector.tensor_reduce(
                ssq, sq3, mybir.AxisListType.X, mybir.AluOpType.add
            )
            sstd = scr.tile([128, HG_], F32, tag="sstd")
            nc.scalar.activation(
                sstd, ssq, AF.Sqrt, bias=eps_t[:, 0:1], scale=1.0 / hd
            )
            rinv = scr.tile([128, HG_], F32, tag="rinv")
            nc.vector.reciprocal(rinv, sstd)
            qn = scr.tile([128, F1], BF16, tag="qn")
            for h in range(HG_):
                nc.scalar.mul(
                    qn[:, h * hd:(h + 1) * hd],
                    ps[:, h * hd:(h + 1) * hd],
                    rinv[:, h:h + 1],
                )
            # RoPE (norm weights pre-folded into the rope tables)
            qn3 = qn.rearrange("p (h two d) -> p h two d", h=HG_, two=2)
            rq = scr.tile([128, F1], BF16, tag="rq")
            rq3 = rq.rearrange("p (h two d) -> p h two d", h=HG_, two=2)
            tmp = scr.tile([128, HG_ * HB], BF16, tag="tmp")
            tm3 = tmp.rearrange("p (h d) -> p h d", h=HG_)
            tmp2 = scr.tile([128, HG_ * HB], BF16, tag="tmp2")
            tm23 = tmp2.rearrange("p (h d) -> p h d", h=HG_)

            def rope_c(c_idx):
                bse = rp[:, i, c_idx * HB:(c_idx + 1) * HB]
                return bass.AP(
                    tensor=bse.tensor,
                    offset=bse.offset,
                    ap=[list(bse.ap[0]), [0, HG_], list(bse.ap[-1])],
                )

            nc.vector.tensor_mul(tm3, qn3[:, :, 0, :], rope_c(0))
            nc.vector.tensor_mul(tm23, qn3[:, :, 1, :], rope_c(1))
            nc.vector.tensor_sub(rq3[:, :, 0, :], tm3, tm23)
            nc.vector.tensor_mul(tm3, qn3[:, :, 1, :], rope_c(2))
            nc.vector.tensor_mul(tm23, qn3[:, :, 0, :], rope_c(3))
            nc.vector.tensor_add(rq3[:, :, 1, :], tm3, tm23)
            # head-transpose q/k (4 heads into one PSUM bank, 1 copy)
            pt = ps_tr.tile([128, HG_ * hd], BF16, tag="pstr")
            for h in range(HG_):
                nc.tensor.transpose(
                    pt[:, h * hd:(h + 1) * hd],
                    rq[:, h * hd:(h + 1) * hd], ident)
            dst = qkT[:, fc, :, ts]
            ptv = pt.rearrange("p (h t) -> p h t", h=HG_)
            if i % 2 == 0:
                nc.vector.tensor_copy(dst, ptv)
            else:
                nc.scalar.copy(dst, ptv)

        # ---------------- attention block (heads subset of one q-block) ----
        yT = {}

        def attn_block(j, heads):
            if j not in yT:
                yT[j] = yP.tile([128, HG_, TQ], BF16, tag="yT",
                                name=f"yT{j}")
            yTj = yT[j]
            nk = R * j + R          # valid 128-wide k tiles (causal)
            npairs = nk // 2
            for h in heads:
                dacc2 = dP.tile([128, 2 * TQ], F16, tag="dacc2")
                plist = []
                for kp in range(npairs):
                    s2 = ps_s.tile([128, 2 * TQ], F32, tag="s2")
                    for half in range(2):
                        k = 2 * kp + half
                        nc.tensor.matmul(
                            s2[:, half * TQ:(half + 1) * TQ],
                            qkT[:, 1, h, k * 128:(k + 1) * 128],
                            qkT[:, 0, h, j * TQ:(j + 1) * TQ],
                            start=True,
                            stop=True,
                        )
                    p2 = pP.tile([128, 2 * TQ], F16, tag="p2")
                    nc.scalar.activation(
                        p2, s2, AF.Exp, scale=sm_scale,
                        bias=ebias_t[:, 0:1],
                    )
                    if kp >= npairs - 2:  # the two diagonal-block pairs
                        r0 = 2 * kp - R * j
                        nc.vector.tensor_mul(
                            p2, p2, masks_t[:, r0 * TQ:(r0 + 2) * TQ]
                        )
                    if kp == 1:
                        nc.vector.tensor_add(dacc2, plist[0], p2)
                    elif kp > 1:
                        nc.vector.tensor_add(dacc2, dacc2, p2)
                    plist.append(p2)
                yps = ps_yp.tile([128, TQ], F32, tag="yps")
                for k in range(nk):
                    nc.tensor.matmul(
                        yps,
                        v_all[:, k, h * hd:(h + 1) * hd],
                        plist[k // 2][:, (k % 2) * TQ:(k % 2 + 1) * TQ],
                        start=(k == 0),
                        stop=(k == nk - 1),
                    )
                dsum = dP.tile([128, TQ], F16, tag="dsum")
                nc.vector.tensor_add(dsum, dacc2[:, :TQ], dacc2[:, TQ:])
                # partition-dim reduction via fp16 ones-matmul
                psr = ps_m.tile([1, TQ], F32, tag="mix")
                nc.tensor.matmul(psr, ones_h[:, 0:1], dsum, start=True, stop=True)
                row = dP.tile([1, TQ], F32, tag="row")
                nc.vector.reciprocal_approx_fast(row, psr)
                row_bf = dP.tile([1, TQ], BF16, tag="row_bf")
                nc.scalar.copy(row_bf, row)
                # broadcast the reciprocal row across partitions (K=1 matmul)
                rbp = ps_m.tile([128, TQ], F32, tag="mix")
                nc.tensor.matmul(rbp, ones_r[0:1, :], row_bf, start=True, stop=True)
                rb = rbP.tile([128, TQ], F32, tag="rb")
                nc.vector.tensor_copy(rb, rbp)
                nc.vector.tensor_mul(yTj[:, h, :], yps, rb)

        def proj_block(j):
            yTj = yT[j]
            for o in range(NO):
                pp = ps_m.tile([128, TQ], F32, tag="mix")
                for ci in range(HG_):
                    nc.tensor.matmul(
                        pp,
                        wp[:, ci, o * 128:(o + 1) * 128],
                        yTj[:, ci, :],
                        start=(ci == 0),
                        stop=(ci == HG_ - 1),
                    )
                ost = oP.tile([128, TQ], BF16, tag="ost")
                if o % 2 == 0:
                    nc.scalar.copy(ost, pp)
                else:
                    nc.vector.tensor_copy(ost, pp)
                # rotate output stores across three rings to shrink drain
                oeng = (nc.sync, nc.gpsimd, nc.scalar)[o % 3]
                oeng.dma_start(
                    outT[o * 128:(o + 1) * 128, j * TQ:(j + 1) * TQ], ost
                )

        # ---------------- emission order -----------------------------------
        # v-pass and q-pass run whole; the k-pass t-blocks interleave with
        # the attention/projection blocks they unblock.
        wt_v = load_wt(2, split=True)
        for i in range(NT):
            if i % NJ == 0:
                xq = load_xq(i // NJ)
            qkv_tile(2, i, wt_v, xq)
        wt_q = load_wt(0)
        load_rope("q", rope_q)
        for i in range(NT):
            if i % NJ == 0:
                xq = load_xq(i // NJ)
            qkv_tile(0, i, wt_q, xq)
        wt_k = load_wt(1)
        load_rope("k", rope_k)
        # phase-2 constants (emitted late so their loads sit behind the
        # per-pass weight streams on the same rings)
        for r in range(R):
            nc.sync.dma_start(
                masks_t[:, r * TQ:(r + 1) * TQ],
                masks_d[r * 128:(r + 1) * 128, :]
            )
        for ci in range(HG_):
            nc.gpsimd.dma_start(wp[:, ci, :], wprojT[ci * 128:(ci + 1) * 128, :])

        def k_block(tb):
            xq = load_xq(tb)
            for i in range(tb * NJ, (tb + 1) * NJ):
                qkv_tile(1, i, wt_k, xq)

        k_block(0)
        k_block(1)
        attn_block(0, range(HG_))
        k_block(2)
        attn_block(1, range(HG_))
        proj_block(0)
        k_block(3)
        attn_block(2, range(HG_))
        proj_block(1)
        attn_block(3, range(2))
        proj_block(2)
        attn_block(3, range(2, HG_))
        proj_block(3)

    nc.compile()
    return nc


def make_host_inputs(x, Wqkv, Wproj, q_norm_w, k_norm_w, rope_cos, rope_sin,
                     T_=T, C_=C, HG_=HG, hd=HD, TQ=512):
    """Build the 8 per-core input maps (sharding done on host)."""
    H_ = Wqkv.shape[0] // (3 * hd)
    tpw = H_ // HG_
    R = TQ // 128
    HB = hd // 2

    def rope_tables(w):
        # out1 = qn1*(cos*w1) - qn2*(sin*w2); out2 = qn2*(cos*w2) + qn1*(sin*w1)
        w1, w2 = w[:HB], w[HB:]
        A = rope_cos * w1[None, :]
        Bt = rope_sin * w2[None, :]
        Ct = rope_cos * w2[None, :]
        D = rope_sin * w1[None, :]
        return np.ascontiguousarray(
            np.concatenate([A, Bt, Ct, D], axis=1).astype(ml_dtypes.bfloat16)
        )

    rope_q_h = rope_tables(np.asarray(q_norm_w, dtype=np.float32))
    rope_k_h = rope_tables(np.asarray(k_norm_w, dtype=np.float32))

    # diagonal causal masks: pattern r: valid when tk + 128*r <= tq
    tk = np.arange(128)[:, None]
    tq = np.arange(TQ)[None, :]
    masks = np.concatenate(
        [(tk + 128 * r <= tq) for r in range(R)], axis=0
    ).astype(np.float16)

    Wqkv = np.asarray(Wqkv, dtype=np.float32)
    Wproj = np.asarray(Wproj, dtype=np.float32)
    x = np.asarray(x, dtype=np.float32)

    # per-batch x^T staged on host (shared by the 4 TP cores of that batch)
    xT_h = [np.ascontiguousarray(x[b].T).astype(ml_dtypes.bfloat16)
            for b in range(x.shape[0])]

    in_maps = []
    for core in range(N_CORES):
        b = core // tpw
        g = core % tpw
        rs = slice(g * HG_ * hd, (g + 1) * HG_ * hd)
        W_shard = np.concatenate(
            [Wqkv[0 * H_ * hd:][rs.start:rs.stop],
             Wqkv[1 * H_ * hd:][rs.start:rs.stop],
             Wqkv[2 * H_ * hd:][rs.start:rs.stop]], axis=0
        )  # [3*F1, C]
        in_maps.append({
            "ident": np.eye(128, dtype=ml_dtypes.bfloat16),
            "xT": xT_h[b],
            "wqkvT": np.ascontiguousarray(W_shard.T).astype(ml_dtypes.bfloat16),
            "wprojT": np.ascontiguousarray(Wproj[:, rs].T).astype(ml_dtypes.bfloat16),
            "rope_q": rope_q_h,
            "rope_k": rope_k_h,
            "masks": masks,
        })
    return in_maps


_NC_CACHE = {}


def run_spmd(inputs, **run_kwargs):
    from concourse.bass_utils import run_bass_kernel_spmd

    x = np.asarray(inputs["x"])
    in_maps = make_host_inputs(
        x, inputs["Wqkv"], inputs["Wproj"], inputs["q_norm_w"],
        inputs["k_norm_w"], inputs["rope_cos"], inputs["rope_sin"],
    )
    if "nc" not in _NC_CACHE:
        _NC_CACHE["nc"] = build_nc()
    nc = _NC_CACHE["nc"]
    res = run_bass_kernel_spmd(nc, in_maps, core_ids=list(range(N_CORES)),
                               **run_kwargs)
    tpw = N_CORES // B
    out = np.zeros((B, T, C), dtype=np.float32)
    for core in range(N_CORES):
        b = core // tpw
        out[b] += res.results[core]["outT"].astype(np.float32).T
    return out, res


def kernel(**inputs):
    out, _ = run_spmd(inputs)
    return out


# revision 21
# speedup vs baseline: 1.1713x; 1.1713x over previous
"""Causal self-attention (QK-RMSNorm + RoPE) Trainium2 kernel.

Sharding (Megatron-style, per the TP-over-heads hint):
  8 cores = 2 (batch) x 4 (head groups of 4 heads).
  Each core computes qkv/attention for its 4 heads on its batch and a partial
  projection output; the host sums the 4 partials per batch (the "all-reduce")
  and transposes (the device emits the output feature-major).

Per-core pipeline (all matmuls bf16/fp16 with fp32 PSUM accumulation):
  phase 1: x^T arrives pre-transposed + bf16 from the host (4 big DMAs);
           qkv = W_shard @ x^T (token-major PSUM) in pass order v, q, k;
           for q/k: fused RMSNorm (one ACT square + DVE segmented reduce +
           ACT rsqrt + per-head ACT scale) + RoPE (norm weights pre-folded
           into the rope tables), PE-transpose to [d, t] layout
  phase 2: per (q-block j, head): scores^T = k^T.T @ q^T, exp on ACT with
           tiles paired into 2-bank PSUM (no max subtraction needed:
           |scores| <= sqrt(hd); exp carries a 1/16 bias for fp16 range),
           causal mask by tile skipping + 4 diagonal masks (fp16, DVE 2x),
           y^T = v.T @ p^T in fp16, denominator via fp16 DVE accumulate +
           GpSimd partition-reduce + DVE reciprocal + GpSimd partition
           broadcast; after each j-block's 4 heads, the projection for that
           block runs (overlaps next j's attention)
"""

import math
from contextlib import ExitStack

import numpy as np
import ml_dtypes

import concourse.bass as bass
import concourse.mybir as mybir
import concourse.tile as tile
from concourse import bacc

F32 = mybir.dt.float32
BF16 = mybir.dt.bfloat16
F16 = mybir.dt.float16
AF = mybir.ActivationFunctionType

# Problem constants (hardcoded; kernel.py must be self-contained)
B, T, C, H, HD = 2, 2048, 2048, 16, 128
N_CORES = 8
DP = 2                 # data-parallel ways (batch)
TPW = N_CORES // DP    # tensor-parallel ways (head groups)
HG = H // TPW          # heads per core
EPS = 1e-6
EXP_BIAS = -math.log(16.0)  # scale exp by 1/16: cancels in softmax, keeps
                            # the fp16 p/denominator chain far from overflow


def build_nc(T_=T, C_=C, HG_=HG, hd=HD, TQ=512, TSPLIT=4):
    NT = T_ // 128       # token tiles
    NCt = C_ // 128      # contraction tiles for qkv
    NJ = T_ // TQ        # query-block tiles
    NO = C_ // 128       # output feature tiles
    R = TQ // 128        # diagonal mask patterns per query block
    F1 = HG_ * hd        # width of one of q/k/v chunks on this core
    HB = hd // 2
    TL = T_ // TSPLIT    # tokens per x^T load chunk
    sm_scale = 1.0 / math.sqrt(hd)

    nc = bacc.Bacc(None, target_bir_lowering=False)
    xT = nc.dram_tensor("xT", [C_, T_], BF16, kind="ExternalInput")
    wqkvT = nc.dram_tensor("wqkvT", [C_, 3 * F1], BF16, kind="ExternalInput")
    wprojT = nc.dram_tensor("wprojT", [F1, C_], BF16, kind="ExternalInput")
    rope_q = nc.dram_tensor("rope_q", [T_, 4 * HB], BF16, kind="ExternalInput")
    rope_k = nc.dram_tensor("rope_k", [T_, 4 * HB], BF16, kind="ExternalInput")
    masks_d = nc.dram_tensor("masks", [R * 128, TQ], F16, kind="ExternalInput")
    ident_d = nc.dram_tensor("ident", [128, 128], BF16, kind="ExternalInput")
    outT = nc.dram_tensor("outT", [C_, T_], BF16, kind="ExternalOutput")

    with tile.TileContext(nc) as tc, ExitStack() as big:
        persist = big.enter_context(tc.tile_pool(name="persist", bufs=1))
        v_all = persist.tile([128, NT, F1], F16, tag="v_all")
        qkT = persist.tile([128, 2, HG_, T_], BF16, tag="qkT")
        ident = persist.tile([128, 128], BF16, tag="ident")
        nc.sync.dma_start(ident, ident_d[:])
        eps_t = persist.tile([128, 1], F32, tag="eps")
        nc.vector.memset(eps_t, EPS)
        ebias_t = persist.tile([128, 1], F32, tag="ebias")
        nc.vector.memset(ebias_t, EXP_BIAS)
        ones_h = persist.tile([128, 1], F16, tag="ones_h")
        nc.vector.memset(ones_h, 1.0)
        ones_r = persist.tile([1, 128], BF16, tag="ones_r")
        nc.vector.memset(ones_r, 1.0)
        # phase-2 constants live in the persistent pool so their loads do
        # not wait on phase-1 SBUF deaths
        masks_t = persist.tile([128, R * TQ], F16, tag="masks")
        wp = persist.tile([128, HG_, C_], BF16, tag="wp")

        # ---------------- phase 1: QKV from host-staged x^T ----------------
        with ExitStack() as ph1:
            xh_pool = ph1.enter_context(tc.tile_pool(name="xT", bufs=1))
            wt_pool = ph1.enter_context(tc.tile_pool(name="wt", bufs=2))
            rope_pool = ph1.enter_context(tc.tile_pool(name="rope", bufs=1))
            scr = ph1.enter_context(tc.tile_pool(name="qkscr", bufs=3))
            ps_qkv = ph1.enter_context(
                tc.tile_pool(name="ps_qkv", bufs=4, space="PSUM"))
            ps_tr = ph1.enter_context(
                tc.tile_pool(name="ps_tr", bufs=2, space="PSUM"))

            rope_sb = {}

            def load_rope(nm, dr):
                t_ = rope_pool.tile([128, NT, 4 * HB], BF16, tag=f"rope{nm}")
                nc.gpsimd.dma_start(t_, dr[:].rearrange("(n p) f -> p n f", p=128))
                rope_sb[nm] = t_

            # W chunks on the sync HWDGE ring; the first (v) pass is gated
            # by W arrival, so split its chunks across sync + gpsimd rings
            # and issue them before the x^T chunks
            def load_wt(fc, split=False):
                wt = []
                for ci in range(NCt):
                    t_ = wt_pool.tile([128, F1], BF16, tag=f"wt{ci}", bufs=2)
                    eng = nc.gpsimd if (split and ci % 2 == 1) else nc.sync
                    eng.dma_start(
                        t_,
                        wqkvT[ci * 128:(ci + 1) * 128, fc * F1:(fc + 1) * F1],
                    )
                    wt.append(t_)
                return wt

            wts = {2: load_wt(2, split=True)}

            # x^T [c, t] staged by the host; 8 t-chunk DMAs. Rings deliver
            # ~110-130 GB/s each and chunk i is consumed at ~6+7i us, so the
            # idle ACT ring takes the first four chunks back-to-back and the
            # W-carrying sync/gpsimd rings take the later ones.
            NXC = 8
            XL = T_ // NXC
            xh = xh_pool.tile([128, NCt, T_], BF16, tag="xh")
            xT_r = xT[:].rearrange("(ci p) t -> p ci t", p=128)
            xh_rings = [nc.scalar, nc.scalar, nc.scalar, nc.scalar,
                        nc.sync, nc.gpsimd, nc.sync, nc.gpsimd]
            for tc_i in range(NXC):
                sl = slice(tc_i * XL, (tc_i + 1) * XL)
                xh_rings[tc_i].dma_start(xh[:, :, sl], xT_r[:, :, sl])

            for fc in (2, 0, 1):  # pass order: v, q, k
                wt = wts.get(fc)
                if wt is None:
                    wt = load_wt(fc)
                if fc == 0:
                    load_rope("q", rope_q)
                elif fc == 1:
                    load_rope("k", rope_k)
                for i in range(NT):
                    ts = slice(i * 128, (i + 1) * 128)
                    ps = ps_qkv.tile([128, F1], F32, tag="psqkv")
                    for ci in range(NCt):
                        nc.tensor.matmul(
                            ps,
                            xh[:, ci, ts],
                            wt[ci],
                            start=(ci == 0),
                            stop=(ci == NCt - 1),
                        )
                    if fc == 2:
                        nc.vector.tensor_copy(v_all[:, i, :], ps)
                        continue
                    rp = rope_sb["q" if fc == 0 else "k"]
                    # RMS norm stats: one ACT square over the whole tile,
                    # per-head sums on DVE, fused rsqrt on ACT
                    sq = scr.tile([128, F1], BF16, tag="sq")
                    nc.scalar.activation(sq, ps, AF.Square)
                    sq3 = sq.rearrange("p (h d) -> p h d", h=HG_)
                    ssq = scr.tile([128, HG_], F32, tag="ssq")
                    nc.vector.tensor_reduce(
                        ssq, sq3, mybir.AxisListType.X, mybir.AluOpType.add
                    )
                    sstd = scr.tile([128, HG_], F32, tag="sstd")
                    nc.scalar.activation(
                        sstd, ssq, AF.Sqrt, bias=eps_t[:, 0:1], scale=1.0 / hd
                    )
                    rinv = scr.tile([128, HG_], F32, tag="rinv")
                    nc.vector.reciprocal(rinv, sstd)
                    qn = scr.tile([128, F1], BF16, tag="qn")
                    for h in range(HG_):
                        nc.scalar.mul(
                            qn[:, h * hd:(h + 1) * hd],
                            ps[:, h * hd:(h + 1) * hd],
                            rinv[:, h:h + 1],
                        )
                    # RoPE (norm weights pre-folded into the rope tables)
                    qn3 = qn.rearrange("p (h two d) -> p h two d", h=HG_, two=2)
                    rq = scr.tile([128, F1], BF16, tag="rq")
                    rq3 = rq.rearrange("p (h two d) -> p h two d", h=HG_, two=2)
                    tmp = scr.tile([128, HG_ * HB], BF16, tag="tmp")
                    tm3 = tmp.rearrange("p (h d) -> p h d", h=HG_)
                    tmp2 = scr.tile([128, HG_ * HB], BF16, tag="tmp2")
                    tm23 = tmp2.rearrange("p (h d) -> p h d", h=HG_)

                    def rope_c(c_idx):
                        bse = rp[:, i, c_idx * HB:(c_idx + 1) * HB]
                        return bass.AP(
                            tensor=bse.tensor,
                            offset=bse.offset,
                            ap=[list(bse.ap[0]), [0, HG_], list(bse.ap[-1])],
                        )

                    nc.vector.tensor_mul(tm3, qn3[:, :, 0, :], rope_c(0))
                    nc.vector.tensor_mul(tm23, qn3[:, :, 1, :], rope_c(1))
                    nc.vector.tensor_sub(rq3[:, :, 0, :], tm3, tm23)
                    nc.vector.tensor_mul(tm3, qn3[:, :, 1, :], rope_c(2))
                    nc.vector.tensor_mul(tm23, qn3[:, :, 0, :], rope_c(3))
                    nc.vector.tensor_add(rq3[:, :, 1, :], tm3, tm23)
                    # head-transpose q/k (4 heads into one PSUM bank, 1 copy)
                    pt = ps_tr.tile([128, HG_ * hd], BF16, tag="pstr")
                    for h in range(HG_):
                        nc.tensor.transpose(
                            pt[:, h * hd:(h + 1) * hd],
                            rq[:, h * hd:(h + 1) * hd], ident)
                    dst = qkT[:, fc, :, ts]
                    ptv = pt.rearrange("p (h t) -> p h t", h=HG_)
                    if i % 2 == 0:
                        nc.vector.tensor_copy(dst, ptv)
                    else:
                        nc.scalar.copy(dst, ptv)

            # phase-2 constants: issue after the per-pass W loads so they do
            # not delay the v/q-pass weight streams on the same rings
            for r in range(R):
                nc.sync.dma_start(
                    masks_t[:, r * TQ:(r + 1) * TQ],
                    masks_d[r * 128:(r + 1) * 128, :]
                )
            for ci in range(HG_):
                nc.gpsimd.dma_start(wp[:, ci, :], wprojT[ci * 128:(ci + 1) * 128, :])

        # -------- phase 2 + 3: attention with interleaved projection --------
        with ExitStack() as ph2:
            pP = ph2.enter_context(tc.tile_pool(name="pP", bufs=12))
            dP = ph2.enter_context(tc.tile_pool(name="dP", bufs=2))
            rbP = ph2.enter_context(tc.tile_pool(name="rbP", bufs=2))
            yP = ph2.enter_context(tc.tile_pool(name="yP", bufs=1))
            oP = ph2.enter_context(tc.tile_pool(name="oP", bufs=4))
            ps_s = ph2.enter_context(tc.tile_pool(name="ps_s", bufs=2, space="PSUM"))
            ps_yp = ph2.enter_context(tc.tile_pool(name="ps_y", bufs=2, space="PSUM"))
            ps_m = ph2.enter_context(tc.tile_pool(name="ps_m", bufs=2, space="PSUM"))

            yT = {}

            def attn_block(j, heads):
                if j not in yT:
                    yT[j] = yP.tile([128, HG_, TQ], BF16, tag=f"yT{j}",
                                    bufs=1, name=f"yT{j}")
                yTj = yT[j]
                nk = R * j + R          # valid 128-wide k tiles (causal)
                npairs = nk // 2
                for h in heads:
                    dacc2 = dP.tile([128, 2 * TQ], F16, tag="dacc2")
                    plist = []
                    for kp in range(npairs):
                        s2 = ps_s.tile([128, 2 * TQ], F32, tag="s2")
                        for half in range(2):
                            k = 2 * kp + half
                            nc.tensor.matmul(
                                s2[:, half * TQ:(half + 1) * TQ],
                                qkT[:, 1, h, k * 128:(k + 1) * 128],
                                qkT[:, 0, h, j * TQ:(j + 1) * TQ],
                                start=True,
                                stop=True,
                            )
                        p2 = pP.tile([128, 2 * TQ], F16, tag="p2")
                        nc.scalar.activation(
                            p2, s2, AF.Exp, scale=sm_scale,
                            bias=ebias_t[:, 0:1],
                        )
                        if kp >= npairs - 2:  # the two diagonal-block pairs
                            r0 = 2 * kp - R * j
                            nc.vector.tensor_mul(
                                p2, p2, masks_t[:, r0 * TQ:(r0 + 2) * TQ]
                            )
                        if kp == 1:
                            nc.vector.tensor_add(dacc2, plist[0], p2)
                        elif kp > 1:
                            nc.vector.tensor_add(dacc2, dacc2, p2)
                        plist.append(p2)
                    yps = ps_yp.tile([128, TQ], F32, tag="yps")
                    for k in range(nk):
                        nc.tensor.matmul(
                            yps,
                            v_all[:, k, h * hd:(h + 1) * hd],
                            plist[k // 2][:, (k % 2) * TQ:(k % 2 + 1) * TQ],
                            start=(k == 0),
                            stop=(k == nk - 1),
                        )
                    dsum = dP.tile([128, TQ], F16, tag="dsum")
                    nc.vector.tensor_add(dsum, dacc2[:, :TQ], dacc2[:, TQ:])
                    # partition-dim reduction via fp16 ones-matmul
                    psr = ps_m.tile([1, TQ], F32, tag="mix")
                    nc.tensor.matmul(psr, ones_h[:, 0:1], dsum, start=True, stop=True)
                    row = dP.tile([1, TQ], F32, tag="row")
                    nc.vector.reciprocal_approx_fast(row, psr)
                    row_bf = dP.tile([1, TQ], BF16, tag="row_bf")
                    nc.scalar.copy(row_bf, row)
                    # broadcast the reciprocal row across partitions (K=1 matmul)
                    rbp = ps_m.tile([128, TQ], F32, tag="mix")
                    nc.tensor.matmul(rbp, ones_r[0:1, :], row_bf, start=True, stop=True)
                    rb = rbP.tile([128, TQ], F32, tag="rb")
                    nc.vector.tensor_copy(rb, rbp)
                    nc.vector.tensor_mul(yTj[:, h, :], yps, rb)
            # projection for a q block (overlaps later attention blocks)
            def proj_block(j):
                yTj = yT[j]
                for o in range(NO):
                    pp = ps_m.tile([128, TQ], F32, tag="mix")
                    for ci in range(HG_):
                        nc.tensor.matmul(
                            pp,
                            wp[:, ci, o * 128:(o + 1) * 128],
                            yTj[:, ci, :],
                            start=(ci == 0),
                            stop=(ci == HG_ - 1),
                        )
                    ost = oP.tile([128, TQ], BF16, tag="ost")
                    if o % 2 == 0:
                        nc.scalar.copy(ost, pp)
                    else:
                        nc.vector.tensor_copy(ost, pp)
                    # rotate output stores across three rings to shrink drain
                    oeng = (nc.sync, nc.gpsimd, nc.scalar)[o % 3]
                    oeng.dma_start(
                        outT[o * 128:(o + 1) * 128, j * TQ:(j + 1) * TQ], ost
                    )

            # split the last (largest) attention block in two so its
            # exp-bound sections overlap the last projections
            attn_block(0, range(HG_))
            proj_block(0)
            attn_block(1, range(HG_))
            proj_block(1)
            attn_block(2, range(HG_))
            attn_block(3, range(2))
            proj_block(2)
            attn_block(3, range(2, HG_))
            proj_block(3)

    nc.compile()
    return nc


def make_host_inputs(x, Wqkv, Wproj, q_norm_w, k_norm_w, rope_cos, rope_sin,
                     T_=T, C_=C, HG_=HG, hd=HD, TQ=512):
    """Build the 8 per-core input maps (sharding done on host)."""
    H_ = Wqkv.shape[0] // (3 * hd)
    tpw = H_ // HG_
    R = TQ // 128
    HB = hd // 2

    def rope_tables(w):
        # out1 = qn1*(cos*w1) - qn2*(sin*w2); out2 = qn2*(cos*w2) + qn1*(sin*w1)
        w1, w2 = w[:HB], w[HB:]
        A = rope_cos * w1[None, :]
        Bt = rope_sin * w2[None, :]
        Ct = rope_cos * w2[None, :]
        D = rope_sin * w1[None, :]
        return np.ascontiguousarray(
            np.concatenate([A, Bt, Ct, D], axis=1).astype(ml_dtypes.bfloat16)
        )

    rope_q_h = rope_tables(np.asarray(q_norm_w, dtype=np.float32))
    rope_k_h = rope_tables(np.asarray(k_norm_w, dtype=np.float32))

    # diagonal causal masks: pattern r: valid when tk + 128*r <= tq
    tk = np.arange(128)[:, None]
    tq = np.arange(TQ)[None, :]
    masks = np.concatenate(
        [(tk + 128 * r <= tq) for r in range(R)], axis=0
    ).astype(np.float16)

    Wqkv = np.asarray(Wqkv, dtype=np.float32)
    Wproj = np.asarray(Wproj, dtype=np.float32)
    x = np.asarray(x, dtype=np.float32)

    # per-batch x^T staged on host (shared by the 4 TP cores of that batch)
    xT_h = [np.ascontiguousarray(x[b].T).astype(ml_dtypes.bfloat16)
            for b in range(x.shape[0])]

    in_maps = []
    for core in range(N_CORES):
        b = core // tpw
        g = core % tpw
        rs = slice(g * HG_ * hd, (g + 1) * HG_ * hd)
        W_shard = np.concatenate(
            [Wqkv[0 * H_ * hd:][rs.start:rs.stop],
             Wqkv[1 * H_ * hd:][rs.start:rs.stop],
             Wqkv[2 * H_ * hd:][rs.start:rs.stop]], axis=0
        )  # [3*F1, C]
        in_maps.append({
            "ident": np.eye(128, dtype=ml_dtypes.bfloat16),
            "xT": xT_h[b],
            "wqkvT": np.ascontiguousarray(W_shard.T).astype(ml_dtypes.bfloat16),
            "wprojT": np.ascontiguousarray(Wproj[:, rs].T).astype(ml_dtypes.bfloat16),
            "rope_q": rope_q_h,
            "rope_k": rope_k_h,
            "masks": masks,
        })
    return in_maps


_NC_CACHE = {}


def run_spmd(inputs, **run_kwargs):
    from concourse.bass_utils import run_bass_kernel_spmd

    x = np.asarray(inputs["x"])
    in_maps = make_host_inputs(
        x, inputs["Wqkv"], inputs["Wproj"], inputs["q_norm_w"],
        inputs["k_norm_w"], inputs["rope_cos"], inputs["rope_sin"],
    )
    if "nc" not in _NC_CACHE:
        _NC_CACHE["nc"] = build_nc()
    nc = _NC_CACHE["nc"]
    res = run_bass_kernel_spmd(nc, in_maps, core_ids=list(range(N_CORES)),
                               **run_kwargs)
    tpw = N_CORES // B
    out = np.zeros((B, T, C), dtype=np.float32)
    for core in range(N_CORES):
        b = core // tpw
        out[b] += res.results[core]["outT"].astype(np.float32).T
    return out, res


def kernel(**inputs):
    out, _ = run_spmd(inputs)
    return out


# revision 22
# speedup vs baseline: 1.1795x; 1.0071x over previous
"""Causal self-attention (QK-RMSNorm + RoPE) Trainium2 kernel.

Sharding (Megatron-style, per the TP-over-heads hint):
  8 cores = 2 (batch) x 4 (head groups of 4 heads).
  Each core computes qkv/attention for its 4 heads on its batch and a partial
  projection output; the host sums the 4 partials per batch (the "all-reduce")
  and transposes (the device emits the output feature-major).

Per-core pipeline (all matmuls bf16/fp16 with fp32 PSUM accumulation):
  phase 1: x^T arrives pre-transposed + bf16 from the host (4 big DMAs);
           qkv = W_shard @ x^T (token-major PSUM) in pass order v, q, k;
           for q/k: fused RMSNorm (one ACT square + DVE segmented reduce +
           ACT rsqrt + per-head ACT scale) + RoPE (norm weights pre-folded
           into the rope tables), PE-transpose to [d, t] layout
  phase 2: per (q-block j, head): scores^T = k^T.T @ q^T, exp on ACT with
           tiles paired into 2-bank PSUM (no max subtraction needed:
           |scores| <= sqrt(hd); exp carries a 1/16 bias for fp16 range),
           causal mask by tile skipping + 4 diagonal masks (fp16, DVE 2x),
           y^T = v.T @ p^T in fp16, denominator via fp16 DVE accumulate +
           GpSimd partition-reduce + DVE reciprocal + GpSimd partition
           broadcast; after each j-block's 4 heads, the projection for that
           block runs (overlaps next j's attention)
"""

import math
from contextlib import ExitStack

import numpy as np
import ml_dtypes

import concourse.bass as bass
import concourse.mybir as mybir
import concourse.tile as tile
from concourse import bacc

F32 = mybir.dt.float32
BF16 = mybir.dt.bfloat16
F16 = mybir.dt.float16
AF = mybir.ActivationFunctionType

# Problem constants (hardcoded; kernel.py must be self-contained)
B, T, C, H, HD = 2, 2048, 2048, 16, 128
N_CORES = 8
DP = 2                 # data-parallel ways (batch)
TPW = N_CORES // DP    # tensor-parallel ways (head groups)
HG = H // TPW          # heads per core
EPS = 1e-6
EXP_BIAS = -math.log(16.0)  # scale exp by 1/16: cancels in softmax, keeps
                            # the fp16 p/denominator chain far from overflow


def build_nc(T_=T, C_=C, HG_=HG, hd=HD, TQ=512, TSPLIT=4):
    NT = T_ // 128       # token tiles
    NCt = C_ // 128      # contraction tiles for qkv
    NJ = T_ // TQ        # query-block tiles
    NO = C_ // 128       # output feature tiles
    R = TQ // 128        # diagonal mask patterns per query block
    F1 = HG_ * hd        # width of one of q/k/v chunks on this core
    HB = hd // 2
    TL = T_ // TSPLIT    # tokens per x^T load chunk
    sm_scale = 1.0 / math.sqrt(hd)

    nc = bacc.Bacc(None, target_bir_lowering=False)
    xT = nc.dram_tensor("xT", [C_, T_], BF16, kind="ExternalInput")
    wqkvT = nc.dram_tensor("wqkvT", [C_, 3 * F1], BF16, kind="ExternalInput")
    wprojT = nc.dram_tensor("wprojT", [F1, C_], BF16, kind="ExternalInput")
    rope_q = nc.dram_tensor("rope_q", [T_, 4 * HB], BF16, kind="ExternalInput")
    rope_k = nc.dram_tensor("rope_k", [T_, 4 * HB], BF16, kind="ExternalInput")
    masks_d = nc.dram_tensor("masks", [R * 128, TQ], F16, kind="ExternalInput")
    ident_d = nc.dram_tensor("ident", [128, 128], BF16, kind="ExternalInput")
    outT = nc.dram_tensor("outT", [C_, T_], BF16, kind="ExternalOutput")

    with tile.TileContext(nc) as tc, ExitStack() as big:
        persist = big.enter_context(tc.tile_pool(name="persist", bufs=1))
        v_all = persist.tile([128, NT, F1], F16, tag="v_all")
        qkT = persist.tile([128, 2, HG_, T_], BF16, tag="qkT")
        ident = persist.tile([128, 128], BF16, tag="ident")
        nc.sync.dma_start(ident, ident_d[:])
        eps_t = persist.tile([128, 1], F32, tag="eps")
        nc.vector.memset(eps_t, EPS)
        ebias_t = persist.tile([128, 1], F32, tag="ebias")
        nc.vector.memset(ebias_t, EXP_BIAS)
        ones_h = persist.tile([128, 1], F16, tag="ones_h")
        nc.vector.memset(ones_h, 1.0)
        ones_r = persist.tile([1, 128], BF16, tag="ones_r")
        nc.vector.memset(ones_r, 1.0)
        # phase-2 constants live in the persistent pool so their loads do
        # not wait on phase-1 SBUF deaths
        masks_t = persist.tile([128, R * TQ], F16, tag="masks")
        wp = persist.tile([128, HG_, C_], BF16, tag="wp")

        # ---------------- phase 1: QKV from host-staged x^T ----------------
        with ExitStack() as ph1:
            xh_pool = ph1.enter_context(tc.tile_pool(name="xT", bufs=1))
            wt_pool = ph1.enter_context(tc.tile_pool(name="wt", bufs=2))
            rope_pool = ph1.enter_context(tc.tile_pool(name="rope", bufs=1))
            scr = ph1.enter_context(tc.tile_pool(name="qkscr", bufs=3))
            ps_qkv = ph1.enter_context(
                tc.tile_pool(name="ps_qkv", bufs=4, space="PSUM"))
            ps_tr = ph1.enter_context(
                tc.tile_pool(name="ps_tr", bufs=2, space="PSUM"))

            rope_sb = {}

            def load_rope(nm, dr):
                t_ = rope_pool.tile([128, NT, 4 * HB], BF16, tag=f"rope{nm}")
                nc.gpsimd.dma_start(t_, dr[:].rearrange("(n p) f -> p n f", p=128))
                rope_sb[nm] = t_

            # W chunks on the sync HWDGE ring; the first (v) pass is gated
            # by W arrival, so split its chunks across sync + gpsimd rings
            # and issue them before the x^T chunks
            def load_wt(fc, split=False):
                wt = []
                for ci in range(NCt):
                    t_ = wt_pool.tile([128, F1], BF16, tag=f"wt{ci}", bufs=2)
                    eng = nc.gpsimd if (split and ci % 2 == 1) else nc.sync
                    eng.dma_start(
                        t_,
                        wqkvT[ci * 128:(ci + 1) * 128, fc * F1:(fc + 1) * F1],
                    )
                    wt.append(t_)
                return wt

            wts = {2: load_wt(2, split=True)}

            # x^T [c, t] staged by the host; 8 t-chunk DMAs. Rings deliver
            # ~110-130 GB/s each and chunk i is consumed at ~6+7i us, so the
            # idle ACT ring takes the first four chunks back-to-back and the
            # W-carrying sync/gpsimd rings take the later ones.
            NXC = 8
            XL = T_ // NXC
            xh = xh_pool.tile([128, NCt, T_], BF16, tag="xh")
            xT_r = xT[:].rearrange("(ci p) t -> p ci t", p=128)
            xh_rings = [nc.scalar, nc.scalar, nc.scalar, nc.scalar,
                        nc.sync, nc.gpsimd, nc.sync, nc.gpsimd]
            for tc_i in range(NXC):
                sl = slice(tc_i * XL, (tc_i + 1) * XL)
                xh_rings[tc_i].dma_start(xh[:, :, sl], xT_r[:, :, sl])

            for fc in (2, 0, 1):  # pass order: v, q, k
                wt = wts.get(fc)
                if wt is None:
                    wt = load_wt(fc)
                if fc == 0:
                    load_rope("q", rope_q)
                elif fc == 1:
                    load_rope("k", rope_k)
                for i in range(NT):
                    ts = slice(i * 128, (i + 1) * 128)
                    ps = ps_qkv.tile([128, F1], F32, tag="psqkv")
                    for ci in range(NCt):
                        nc.tensor.matmul(
                            ps,
                            xh[:, ci, ts],
                            wt[ci],
                            start=(ci == 0),
                            stop=(ci == NCt - 1),
                        )
                    if fc == 2:
                        nc.vector.tensor_copy(v_all[:, i, :], ps)
                        continue
                    rp = rope_sb["q" if fc == 0 else "k"]
                    # RMS norm stats: one ACT square over the whole tile,
                    # per-head sums on DVE, fused rsqrt on ACT
                    sq = scr.tile([128, F1], BF16, tag="sq")
                    nc.scalar.activation(sq, ps, AF.Square)
                    sq3 = sq.rearrange("p (h d) -> p h d", h=HG_)
                    ssq = scr.tile([128, HG_], F32, tag="ssq")
                    nc.vector.tensor_reduce(
                        ssq, sq3, mybir.AxisListType.X, mybir.AluOpType.add
                    )
                    sstd = scr.tile([128, HG_], F32, tag="sstd")
                    nc.scalar.activation(
                        sstd, ssq, AF.Sqrt, bias=eps_t[:, 0:1], scale=1.0 / hd
                    )
                    rinv = scr.tile([128, HG_], F32, tag="rinv")
                    nc.vector.reciprocal(rinv, sstd)
                    qn = scr.tile([128, F1], BF16, tag="qn")
                    for h in range(HG_):
                        nc.scalar.mul(
                            qn[:, h * hd:(h + 1) * hd],
                            ps[:, h * hd:(h + 1) * hd],
                            rinv[:, h:h + 1],
                        )
                    # RoPE (norm weights pre-folded into the rope tables)
                    qn3 = qn.rearrange("p (h two d) -> p h two d", h=HG_, two=2)
                    rq = scr.tile([128, F1], BF16, tag="rq")
                    rq3 = rq.rearrange("p (h two d) -> p h two d", h=HG_, two=2)
                    tmp = scr.tile([128, HG_ * HB], BF16, tag="tmp")
                    tm3 = tmp.rearrange("p (h d) -> p h d", h=HG_)
                    tmp2 = scr.tile([128, HG_ * HB], BF16, tag="tmp2")
                    tm23 = tmp2.rearrange("p (h d) -> p h d", h=HG_)

                    def rope_c(c_idx):
                        bse = rp[:, i, c_idx * HB:(c_idx + 1) * HB]
                        return bass.AP(
                            tensor=bse.tensor,
                            offset=bse.offset,
                            ap=[list(bse.ap[0]), [0, HG_], list(bse.ap[-1])],
                        )

                    nc.vector.tensor_mul(tm3, qn3[:, :, 0, :], rope_c(0))
                    nc.vector.tensor_mul(tm23, qn3[:, :, 1, :], rope_c(1))
                    nc.vector.tensor_sub(rq3[:, :, 0, :], tm3, tm23)
                    nc.vector.tensor_mul(tm3, qn3[:, :, 1, :], rope_c(2))
                    nc.vector.tensor_mul(tm23, qn3[:, :, 0, :], rope_c(3))
                    nc.vector.tensor_add(rq3[:, :, 1, :], tm3, tm23)
                    # head-transpose q/k (4 heads into one PSUM bank, 1 copy)
                    pt = ps_tr.tile([128, HG_ * hd], BF16, tag="pstr")
                    for h in range(HG_):
                        nc.tensor.transpose(
                            pt[:, h * hd:(h + 1) * hd],
                            rq[:, h * hd:(h + 1) * hd], ident)
                    dst = qkT[:, fc, :, ts]
                    ptv = pt.rearrange("p (h t) -> p h t", h=HG_)
                    if i % 2 == 0:
                        nc.vector.tensor_copy(dst, ptv)
                    else:
                        nc.scalar.copy(dst, ptv)

            # phase-2 constants: issue after the per-pass W loads so they do
            # not delay the v/q-pass weight streams on the same rings
            for r in range(R):
                nc.sync.dma_start(
                    masks_t[:, r * TQ:(r + 1) * TQ],
                    masks_d[r * 128:(r + 1) * 128, :]
                )
            for ci in range(HG_):
                nc.gpsimd.dma_start(wp[:, ci, :], wprojT[ci * 128:(ci + 1) * 128, :])

        # -------- phase 2 + 3: attention with interleaved projection --------
        with ExitStack() as ph2:
            pP = ph2.enter_context(tc.tile_pool(name="pP", bufs=12))
            dP = ph2.enter_context(tc.tile_pool(name="dP", bufs=2))
            rbP = ph2.enter_context(tc.tile_pool(name="rbP", bufs=2))
            yP = ph2.enter_context(tc.tile_pool(name="yP", bufs=1))
            oP = ph2.enter_context(tc.tile_pool(name="oP", bufs=4))
            ps_s = ph2.enter_context(tc.tile_pool(name="ps_s", bufs=2, space="PSUM"))
            ps_yp = ph2.enter_context(tc.tile_pool(name="ps_y", bufs=2, space="PSUM"))
            ps_m = ph2.enter_context(tc.tile_pool(name="ps_m", bufs=2, space="PSUM"))

            for j in range(NJ):
                yTj = yP.tile([128, HG_, TQ], BF16, tag=f"yT{j}", bufs=1)
                nk = R * j + R          # valid 128-wide k tiles (causal)
                npairs = nk // 2
                for h in range(HG_):
                    dacc2 = dP.tile([128, 2 * TQ], F16, tag="dacc2")
                    plist = []
                    for kp in range(npairs):
                        s2 = ps_s.tile([128, 2 * TQ], F32, tag="s2")
                        for half in range(2):
                            k = 2 * kp + half
                            nc.tensor.matmul(
                                s2[:, half * TQ:(half + 1) * TQ],
                                qkT[:, 1, h, k * 128:(k + 1) * 128],
                                qkT[:, 0, h, j * TQ:(j + 1) * TQ],
                                start=True,
                                stop=True,
                            )
                        p2 = pP.tile([128, 2 * TQ], F16, tag="p2")
                        nc.scalar.activation(
                            p2, s2, AF.Exp, scale=sm_scale,
                            bias=ebias_t[:, 0:1],
                        )
                        if kp >= npairs - 2:  # the two diagonal-block pairs
                            r0 = 2 * kp - R * j
                            nc.vector.tensor_mul(
                                p2, p2, masks_t[:, r0 * TQ:(r0 + 2) * TQ]
                            )
                        if kp == 1:
                            nc.vector.tensor_add(dacc2, plist[0], p2)
                        elif kp > 1:
                            nc.vector.tensor_add(dacc2, dacc2, p2)
                        plist.append(p2)
                    yps = ps_yp.tile([128, TQ], F32, tag="yps")
                    for k in range(nk):
                        nc.tensor.matmul(
                            yps,
                            v_all[:, k, h * hd:(h + 1) * hd],
                            plist[k // 2][:, (k % 2) * TQ:(k % 2 + 1) * TQ],
                            start=(k == 0),
                            stop=(k == nk - 1),
                        )
                    dsum = dP.tile([128, TQ], F16, tag="dsum")
                    nc.vector.tensor_add(dsum, dacc2[:, :TQ], dacc2[:, TQ:])
                    # partition-dim reduction via fp16 ones-matmul
                    psr = ps_m.tile([1, TQ], F32, tag="mix")
                    nc.tensor.matmul(psr, ones_h[:, 0:1], dsum, start=True, stop=True)
                    row = dP.tile([1, TQ], F32, tag="row")
                    nc.vector.reciprocal_approx_fast(row, psr)
                    row_bf = dP.tile([1, TQ], BF16, tag="row_bf")
                    nc.scalar.copy(row_bf, row)
                    # broadcast the reciprocal row across partitions (K=1 matmul)
                    rbp = ps_m.tile([128, TQ], F32, tag="mix")
                    nc.tensor.matmul(rbp, ones_r[0:1, :], row_bf, start=True, stop=True)
                    rb = rbP.tile([128, TQ], F32, tag="rb")
                    nc.vector.tensor_copy(rb, rbp)
                    nc.vector.tensor_mul(yTj[:, h, :], yps, rb)
                # projection for this q block (overlaps next block's attention)
                for o in range(NO):
                    pp = ps_m.tile([128, TQ], F32, tag="mix")
                    for ci in range(HG_):
                        nc.tensor.matmul(
                            pp,
                            wp[:, ci, o * 128:(o + 1) * 128],
                            yTj[:, ci, :],
                            start=(ci == 0),
                            stop=(ci == HG_ - 1),
                        )
                    ost = oP.tile([128, TQ], BF16, tag="ost")
                    if o % 2 == 0:
                        nc.scalar.copy(ost, pp)
                    else:
                        nc.vector.tensor_copy(ost, pp)
                    # rotate output stores across three rings to shrink drain
                    oeng = (nc.sync, nc.gpsimd, nc.scalar)[o % 3]
                    oeng.dma_start(
                        outT[o * 128:(o + 1) * 128, j * TQ:(j + 1) * TQ], ost
                    )

    nc.compile()
    return nc


def make_host_inputs(x, Wqkv, Wproj, q_norm_w, k_norm_w, rope_cos, rope_sin,
                     T_=T, C_=C, HG_=HG, hd=HD, TQ=512):
    """Build the 8 per-core input maps (sharding done on host)."""
    H_ = Wqkv.shape[0] // (3 * hd)
    tpw = H_ // HG_
    R = TQ // 128
    HB = hd // 2

    def rope_tables(w):
        # out1 = qn1*(cos*w1) - qn2*(sin*w2); out2 = qn2*(cos*w2) + qn1*(sin*w1)
        w1, w2 = w[:HB], w[HB:]
        A = rope_cos * w1[None, :]
        Bt = rope_sin * w2[None, :]
        Ct = rope_cos * w2[None, :]
        D = rope_sin * w1[None, :]
        return np.ascontiguousarray(
            np.concatenate([A, Bt, Ct, D], axis=1).astype(ml_dtypes.bfloat16)
        )

    rope_q_h = rope_tables(np.asarray(q_norm_w, dtype=np.float32))
    rope_k_h = rope_tables(np.asarray(k_norm_w, dtype=np.float32))

    # diagonal causal masks: pattern r: valid when tk + 128*r <= tq
    tk = np.arange(128)[:, None]
    tq = np.arange(TQ)[None, :]
    masks = np.concatenate(
        [(tk + 128 * r <= tq) for r in range(R)], axis=0
    ).astype(np.float16)

    Wqkv = np.asarray(Wqkv, dtype=np.float32)
    Wproj = np.asarray(Wproj, dtype=np.float32)
    x = np.asarray(x, dtype=np.float32)

    # per-batch x^T staged on host (shared by the 4 TP cores of that batch)
    xT_h = [np.ascontiguousarray(x[b].T).astype(ml_dtypes.bfloat16)
            for b in range(x.shape[0])]

    in_maps = []
    for core in range(N_CORES):
        b = core // tpw
        g = core % tpw
        rs = slice(g * HG_ * hd, (g + 1) * HG_ * hd)
        W_shard = np.concatenate(
            [Wqkv[0 * H_ * hd:][rs.start:rs.stop],
             Wqkv[1 * H_ * hd:][rs.start:rs.stop],
             Wqkv[2 * H_ * hd:][rs.start:rs.stop]], axis=0
        )  # [3*F1, C]
        in_maps.append({
            "ident": np.eye(128, dtype=ml_dtypes.bfloat16),
            "xT": xT_h[b],
            "wqkvT": np.ascontiguousarray(W_shard.T).astype(ml_dtypes.bfloat16),
            "wprojT": np.ascontiguousarray(Wproj[:, rs].T).astype(ml_dtypes.bfloat16),
            "rope_q": rope_q_h,
            "rope_k": rope_k_h,
            "masks": masks,
        })
    return in_maps


_NC_CACHE = {}


def run_spmd(inputs, **run_kwargs):
    from concourse.bass_utils import run_bass_kernel_spmd

    x = np.asarray(inputs["x"])
    in_maps = make_host_inputs(
        x, inputs["Wqkv"], inputs["Wproj"], inputs["q_norm_w"],
        inputs["k_norm_w"], inputs["rope_cos"], inputs["rope_sin"],
    )
    if "nc" not in _NC_CACHE:
        _NC_CACHE["nc"] = build_nc()
    nc = _NC_CACHE["nc"]
    res = run_bass_kernel_spmd(nc, in_maps, core_ids=list(range(N_CORES)),
                               **run_kwargs)
    tpw = N_CORES // B
    out = np.zeros((B, T, C), dtype=np.float32)
    for core in range(N_CORES):
        b = core // tpw
        out[b] += res.results[core]["outT"].astype(np.float32).T
    return out, res


def kernel(**inputs):
    out, _ = run_spmd(inputs)
    return out
